# revision 38
# baseline (speedup 1.0000x reference)
"""Trainium2 Bass kernel for nn_Attention_21715354649378.

Reference computation (per batch b of 4):
    qkv = w_qkv @ x        x: [256, 4096(=64x64)]   w_qkv: [384, 256]
    q,k,v: [4 heads, 32, 4096];  q *= 32**-0.5
    sim_h = q_h^T k_h   [4096, 4096];  attn = softmax(sim, axis=-1)
    out_h = attn @ v_h^T    -> [4096, 32]
    out = w_out @ concat_heads + b_out   [256, 4096]

Sharding: 8 cores = 4 batches x 2 query-halves. Each core computes K/V for
its full batch plus attention + output projection for its half of the query
pixels. Outputs are disjoint slices -> no collectives.

The limiting resource (cost model): every sim element must transit
PSUM(fp32) -> SBUF through ACT or DVE at 1 elem/cycle/partition, so the
exp stage is organized as ONE fat [128, 1024] op per key-tile pair to
amortize the fixed access latency, with the work split A(CT)/S/T per the
EXP_PAT so ACT, DVE and Pool all stay busy:

  A: ACT activation Exp -> e4m3                      (exact + fp8 quant)
  T: DVE Schraudolph -> bf16 bits; PV runs bf16      (best, costs PE)
  S: DVE Schraudolph -> bf16 bits; Pool copy -> e4m3
  D: DVE Schraudolph -> e4m3 bits (uint8)            (coarse; unused)

Per-core algorithm (keys-in-partition layout; probs are exp(sim)*2^-4 so
they fit fp8-e4m3 range, the scale cancels in softmax normalization):

  ksb[32h+d, key] = W_k x, qsb[32h+d, q] = s W_q x_q   bf16
  vsbT = e4m3 v, keys in partitions, halves interleaved per key-tile pair
      for fp8 DoubleRow; a ones column per 48-block makes the PV matmul
      emit the softmax denominator as output row 32.

  per (h, ci) over 16 key-tile pairs:
    fat psum [128, 1024] <- 2 QK matmuls (contract 32, band 32h)
    probs [128, 1024] via one exp op (A on ACT / T,S on DVE, Pool converts S)
    pv[48, 512] += [v8|1|0..]^T probs   one fp8 DoubleRow matmul per pair
        (or 2 bf16 matmuls for T-pairs)
  norm: rec = recip(pv[32]) direct from PSUM; DRAM-bounce broadcast to 32
        partitions; outh[ci][32h..] = pv[0:32] * bc
  out[ci] = W_o @ outh[ci] + b_out -> DMA out

PSUM: 3x fat qk staging [128, 1024] (also used by projections / output
matmul staging) + 2x pv [128, 512].
"""

import os

import numpy as np
import ml_dtypes

ABLATE = os.environ.get("KERNEL_ABLATE", "")

import concourse.bass as bass
import concourse.mybir as mybir
import concourse.tile as tile
from concourse import bacc
from concourse.bass import ts, ds
from concourse.bass_utils import run_bass_kernel_spmd

HEADS = 4
D = 32
HID = 128
C = 256
N = 4096
NQ = 2048
SCALE = D ** -0.5
NCORES = 8

F32 = mybir.dt.float32
F32R = mybir.dt.float32r
BF16 = mybir.dt.bfloat16
E4 = mybir.dt.float8e4
I16 = mybir.dt.int16
U8 = mybir.dt.uint8
EXP = mybir.ActivationFunctionType.Exp
COPY = mybir.ActivationFunctionType.Copy
IDENT = mybir.ActivationFunctionType.Identity
DR = mybir.MatmulPerfMode.DoubleRow
MUL = mybir.AluOpType.mult
ADD = mybir.AluOpType.add

NKT = N // 128    # 32 key tiles
NPAIR = NKT // 2  # 16 key-tile pairs
NCH = NQ // 512   # 4 query chunks

# probs = exp(sim) * 2^-4  (fits e4m3; cancels in normalization)
LN2_4 = float(4 * np.log(2))
# Schraudolph bf16 bits: exp(x)*2^-4 ~= bf16(int16(x*184.665 + 15736))
SCH_A16 = 184.6650
SCH_B16 = 128.0 * (127 - 4) - 8.0
# Schraudolph e4m3 bits: exp(x)*2^-4 ~= e4m3(uint8(x*11.5416 + 23.6))
SCH_A8 = 11.541560
SCH_B8 = 8.0 * 3 - 0.4

# Engine split over all 256 pairs: A=130 (ACT), S=126 (DVE + casting-DMA
# convert). Strict near-alternation keeps the in-order PE / 3-slot PSUM
# rotation from chaining one engine's stalls into the other's.
def _mk_pat(nA, nS):
    out, a, s = [], 0, 0
    for _ in range(nA + nS):
        if a * nS <= s * nA and a < nA:
            out.append("A")
            a += 1
        else:
            out.append("S")
            s += 1
    return "".join(out)


EXP_PAT = _mk_pat(135, 121)
assert len(EXP_PAT) == 256
PVLAG = 10  # PV trails its exp by this many pairs (covers the cast DMA)


def build_nc(debug=False):
    nc = bacc.Bacc("TRN2")

    dbg = {}
    if debug:
        dbg["ksb"] = nc.declare_dram_parameter("dbg_ksb", [128, N], BF16, isOutput=True)
        dbg["qsb0"] = nc.declare_dram_parameter("dbg_qsb0", [128, 512], BF16, isOutput=True)
        dbg["vsbT"] = nc.declare_dram_parameter("dbg_vsbT", [128, NPAIR * 384], U8, isOutput=True)
        dbg["probs0"] = nc.declare_dram_parameter("dbg_probs0", [128, 1024], U8, isOutput=True)
        dbg["pv0"] = nc.declare_dram_parameter("dbg_pv0", [128, 512], F32, isOutput=True)
        dbg["outh0"] = nc.declare_dram_parameter("dbg_outh0", [HID, 512], F32, isOutput=True)

    xb = nc.declare_dram_parameter("xb", [C, N], BF16, isOutput=False)
    xq = nc.declare_dram_parameter("xq", [C, NQ], BF16, isOutput=False)
    wq4 = nc.declare_dram_parameter("wq4", [C, HID], BF16, isOutput=False)
    wk4 = nc.declare_dram_parameter("wk4", [C, HID], BF16, isOutput=False)
    wv4 = nc.declare_dram_parameter("wv4", [C, HID], BF16, isOutput=False)
    woT = nc.declare_dram_parameter("woT", [HID, C], F32R, isOutput=False)
    bout = nc.declare_dram_parameter("bout", [C, 1], F32, isOutput=False)
    out = nc.declare_dram_parameter("out", [C, NQ], F32, isOutput=True)

    with tile.TileContext(nc) as tc:
        with (
            nc.allow_low_precision(reason="bf16 qk / fp8 pv attention core"),
            tc.tile_pool(name="persist", bufs=1) as persist,
            tc.tile_pool(name="wts", bufs=1) as wts,
        ):
            # ---- persistent SBUF ----
            x_sb = [
                [
                    persist.tile([128, 1024], BF16, tag=f"x{i}{j}", name=f"x{i}{j}")
                    for j in range(4)
                ]
                for i in range(2)
            ]
            xq_sb = [
                [
                    persist.tile([128, 1024], BF16, tag=f"xq{i}{j}", name=f"xq{i}{j}")
                    for j in range(2)
                ]
                for i in range(2)
            ]
            ksb = persist.tile([128, N], BF16, tag="ksb", name="ksb")
            qsb = [
                persist.tile([128, 512], BF16, tag=f"qsb{ci}", name=f"qsb{ci}")
                for ci in range(NCH)
            ]
            # per pair: 4 heads x 2 halves x 48 cols ([v8 (32) | ones | 0s]);
            # single tensor so V evacuations can write 4 pairs in one op.
            # DR stationary M must be a multiple of 16; engine PSUM reads
            # must start 32-aligned, so the denominator row sits at row 32.
            vsbT = persist.tile([128, NPAIR * 384], E4, tag="vsbT", name="vsbT")

            wq_sb = [
                wts.tile([128, HID], BF16, tag=f"wq{i}", name=f"wq{i}")
                for i in range(2)
            ]
            wk_sb = [
                wts.tile([128, HID], BF16, tag=f"wk{i}", name=f"wk{i}")
                for i in range(2)
            ]
            wv_sb = [
                wts.tile([128, HID], BF16, tag=f"wv{i}", name=f"wv{i}")
                for i in range(2)
            ]
            wo_sb = wts.tile([HID, C], F32R, tag="wo")
            bo_sb = [
                wts.tile([128, 1], F32, tag=f"bo{i}", name=f"bo{i}")
                for i in range(2)
            ]
            ebias = wts.tile([128, 1], F32, tag="ebias")

            # ---- input DMAs, ordered so K/Q projections can start earliest ----
            for i in range(2):
                nc.sync.dma_start(out=wk_sb[i][:], in_=wk4[ds(i * 128, 128), :])
            for i in range(2):
                nc.sync.dma_start(
                    out=x_sb[i][0][:], in_=xb[ds(i * 128, 128), ts(0, 1024)]
                )
            for i in range(2):
                nc.sync.dma_start(out=wq_sb[i][:], in_=wq4[ds(i * 128, 128), :])
            for i in range(2):
                nc.sync.dma_start(
                    out=xq_sb[i][0][:], in_=xq[ds(i * 128, 128), ts(0, 1024)]
                )
            for i in range(2):
                nc.sync.dma_start(out=wv_sb[i][:], in_=wv4[ds(i * 128, 128), :])
            for j in range(1, 4):
                for i in range(2):
                    nc.sync.dma_start(
                        out=x_sb[i][j][:], in_=xb[ds(i * 128, 128), ts(j, 1024)]
                    )
            for i in range(2):
                nc.sync.dma_start(
                    out=xq_sb[i][1][:], in_=xq[ds(i * 128, 128), ts(1, 1024)]
                )
            nc.sync.dma_start(out=wo_sb[:], in_=woT[:, :])
            for i in range(2):
                nc.sync.dma_start(out=bo_sb[i][:], in_=bout[ds(i * 128, 128), :])
            nc.vector.memset(ebias[:], -LN2_4)
            # v8 copies cover cols 0:32 of each 48-block; init only the
            # ones (col 32, denominator row) and zero-pad (cols 33:48)
            blk = vsbT[:].rearrange("q (p h hf x) -> q p h hf x", h=4, hf=2, x=48)
            nc.gpsimd.memset(blk[:, :, :, :, 32:33], 1.0)
            nc.gpsimd.memset(blk[:, :, :, :, 33:48], 0.0)

            def x_ap(ct, c0, length):
                t = c0 // 1024
                return x_sb[ct][t][:, ds(c0 % 1024, length)]

            def xq_ap(ct, c0, length):
                t = c0 // 1024
                return xq_sb[ct][t][:, ds(c0 % 1024, length)]

            with (
                tc.tile_pool(name="qkp", bufs=3, space="PSUM") as qkp,
                tc.tile_pool(name="pvp", bufs=1, space="PSUM") as pvp,
                tc.tile_pool(name="probs", bufs=22) as probs_pool,
                tc.tile_pool(name="norm", bufs=4) as norm_pool,
                tc.tile_pool(name="osb", bufs=2) as osb,
                tc.tile_pool(name="dram", bufs=2, space="DRAM") as dram_pool,
            ):
                def fat_tile():
                    return qkp.tile([128, 1024], F32, tag="qk", name="qk")

                # ---- projections (ride the fat psum rotation) ----
                def emit_k2(j2):  # j2 in 0..3, 1024-key chunk
                    ps = fat_tile()
                    for half in range(2):
                        for ct in range(2):
                            nc.tensor.matmul(
                                ps[:, ts(half, 512)],
                                wk_sb[ct][:],
                                x_ap(ct, (2 * j2 + half) * 512, 512),
                                start=(ct == 0),
                                stop=(ct == 1),
                            )
                    nc.scalar.activation(ksb[:, ts(j2, 1024)], ps[:], COPY)

                def emit_q(ci):
                    ps = fat_tile()[:, 0:512]
                    for ct in range(2):
                        nc.tensor.matmul(
                            ps,
                            wq_sb[ct][:],
                            xq_ap(ct, ci * 512, 512),
                            start=(ct == 0),
                            stop=(ct == 1),
                        )
                    nc.scalar.activation(qsb[ci][:], ps, COPY)

                def emit_v4(q0):  # quad q0 in 0..3: key tiles 8q0..8q0+7
                    ps = fat_tile()
                    for t in range(8):
                        for ct in range(2):
                            nc.tensor.matmul(
                                ps[:, ts(t, HID)],
                                x_ap(ct, (8 * q0 + t) * 128, 128),
                                wv_sb[ct][:],
                                start=(ct == 0),
                                stop=(ct == 1),
                            )
                    # evac: psum [kt(4 pairs x 2 hf), h, d] -> vsbT, one op
                    # per hf half (ISA engine APs allow at most 3 free dims)
                    src5 = ps[:].rearrange(
                        "q (tp thf h x) -> q tp thf h x", tp=4, thf=2, h=4
                    )
                    dst5 = vsbT[
                        :, ds(q0 * 4 * 384, 4 * 384)
                    ].rearrange(
                        "q (tp h thf x) -> q tp thf h x", tp=4, h=4, thf=2
                    )[:, :, :, :, 0:32]
                    for hf in range(2):
                        nc.scalar.activation(
                            dst5[:, :, hf], src5[:, :, hf], COPY
                        )

                outh = [
                    osb.tile([HID, 512], F32R, tag=f"outh{c}", name=f"outh{c}")
                    for c in range(NCH)
                ]

                # ---- exp paths (one fat op per pair) ----
                def exp_alloc(kind):
                    if kind == "A":
                        return probs_pool.tile([128, 1024], E4, tag="pr", name="prA")
                    if kind == "D":
                        return probs_pool.tile([128, 1024], U8, tag="pr", name="prD")
                    return probs_pool.tile([128, 1024], I16, tag="pr", name="prT")

                def exp_op(kind, pr, fat):
                    if kind == "A":
                        nc.scalar.activation(pr[:], fat[:], EXP, bias=ebias[:, 0:1])
                    elif kind == "D":
                        nc.vector.tensor_scalar(
                            pr[:], fat[:], SCH_A8, SCH_B8, MUL, ADD
                        )
                    else:
                        nc.vector.tensor_scalar(
                            pr[:], fat[:], SCH_A16, SCH_B16, MUL, ADD
                        )

                def exp_fini(kind, pb):
                    if kind != "S":
                        return pb
                    # S: SWDGE casting DMA converts the bf16 bits to e4m3
                    # (SBUF->SBUF; only gpsimd-initiated DMAs can cast)
                    pr = probs_pool.tile([128, 1024], E4, tag="pr", name="prS")
                    nc.gpsimd.dma_start(out=pr[:], in_=pb.bitcast(BF16)[:])
                    return pr

                # ---- normalization ----
                def emit_recip(h, ci, pv):
                    # den row PSUM->SBUF on ACT (balances DVE norm work),
                    # recip on DVE, then DRAM-bounce broadcast of 1/den.
                    r0, c0 = pv
                    den = norm_pool.tile([1, 512], F32, tag="den", name="den")
                    nc.scalar.activation(
                        den[:], pvar[ds(r0 + 32, 1), ds(c0, 512)], COPY
                    )
                    rec = norm_pool.tile([1, 512], F32, tag="rec", name="rec")
                    nc.vector.reciprocal_approx_fast(rec[:], den[:])
                    rdr = dram_pool.tile([1, 512], F32, tag="rdr", name="rdr")
                    nc.sync.dma_start(out=rdr[:], in_=rec[:])
                    bc = norm_pool.tile([D, 512], F32, tag="bc", name="bc")
                    nc.sync.dma_start(
                        out=bc[:],
                        in_=bass.AP(
                            tensor=rdr.tensor,
                            offset=rdr.offset,
                            ap=[[0, D]] + [list(a) for a in rdr.ap[1:]],
                        ),
                    )
                    return bc

                def emit_norm(h, ci, pv, bc):
                    # evacuation fused with normalization: outh = pv * (1/den)
                    r0, c0 = pv
                    nc.vector.tensor_mul(
                        outh[ci][ds(32 * h, 32), :],
                        pvar[ds(r0, 32), ds(c0, 512)],
                        bc[:],
                    )

                def emit_outproj(ci):
                    op = fat_tile()
                    for ot in range(2):
                        nc.tensor.matmul(
                            op[:, ts(ot, 512)],
                            wo_sb[:, ts(ot, 128)],
                            outh[ci][:],
                            start=True,
                            stop=True,
                        )
                    for ot in range(2):
                        ob = osb.tile([128, 512], F32, tag="ob", name="ob")
                        nc.scalar.activation(
                            ob[:], op[:, ts(ot, 512)], IDENT, bias=bo_sb[ot][:, 0:1]
                        )
                        nc.sync.dma_start(
                            out=out[ds(ot * 128, 128), ts(ci, 512)], in_=ob[:]
                        )

                # ---- prologue ----
                emit_k2(0)
                emit_q(0)
                emit_k2(1)
                emit_v4(0)
                emit_k2(2)
                emit_k2(3)
                vdone = 1

                pending = []   # (kind, probs, pair, h, ci, pv)
                deferred = []  # ci ready for out-projection
                normq = []     # (h, ci, pv) pending denominator recip
                normq2 = []    # (h, ci, pv, bc) pending normalization
                normd = [0] * NCH  # per-ci count of emitted norm-muls

                def pop_pv():
                    kind, probs, p, h, ci, pv = pending.pop(0)
                    r0, c0 = pv
                    pvap = pvar[ds(r0, 48), ds(c0, 512)]
                    vv = vsbT[:, ds(384 * p + 96 * h, 96)]
                    if kind == "T":
                        # bf16 probs: one plain matmul per key tile
                        prb = probs.bitcast(BF16)
                        for t in range(2):
                            nc.tensor.matmul(
                                pvap,
                                vv[:, ds(48 * t, 48)],
                                prb[:, ts(t, 512)],
                                start=(p == 0 and t == 0),
                                stop=(p == NPAIR - 1 and t == 1),
                            )
                    else:
                        prb = probs.bitcast(E4) if kind == "D" else probs
                        nc.tensor.matmul(
                            pvap,
                            vv.rearrange("q (hf m) -> q hf m", hf=2),
                            prb[:].rearrange("q (hf n) -> q hf n", hf=2),
                            start=(p == 0),
                            stop=(p == NPAIR - 1),
                            perf_mode=DR,
                        )
                    if p == NPAIR - 1 and ABLATE != "nonorm":
                        normq.append((h, ci, pv))
                        if h == HEADS - 1:
                            deferred.append(ci)

                # pv arena: 2 banks, alternating per block; with recip at
                # g11 and norm-mul at g1 of the next block, a block's pv is
                # fully read ~10 pairs before its bank is re-written.
                pvar = pvp.tile([128, 1024], F32, tag="pvar", name="pvar")

                _gexp = [0]
                for h in range(HEADS):
                    for ci in range(NCH):
                        blk = h * NCH + ci
                        pv = (0, 512 * (blk % 2))
                        for g in range(NPAIR):
                            kind = EXP_PAT[_gexp[0] % len(EXP_PAT)]
                            _gexp[0] += 1
                            pr0 = exp_alloc(kind)
                            fat = fat_tile()
                            for t in range(2):
                                kt = 2 * g + t
                                nc.tensor.matmul(
                                    fat[:, ts(t, 512)],
                                    ksb[ds(32 * h, 32), ts(kt, 128)],
                                    qsb[ci][ds(32 * h, 32), :],
                                    start=True,
                                    stop=True,
                                    tile_position=(32 * h, 0),
                                )
                            exp_op(kind, pr0, fat)
                            pr = exp_fini(kind, pr0)
                            if debug and h == 0 and ci == 0 and g == 0:
                                nc.sync.dma_start(
                                    out=dbg["probs0"][:, :],
                                    in_=pr[:].bitcast(U8),
                                )
                            pending.append((kind, pr, g, h, ci, pv))
                            if ABLATE == "exponly":
                                pending.pop()
                            while len(pending) > PVLAG:
                                pop_pv()
                            # interleave deferred work into the PE stream
                            if g % 2 == 1 and vdone < 4:
                                emit_v4(vdone)
                                vdone += 1
                            if g == 2 and h == 0 and ci < NCH - 1:
                                emit_q(ci + 1)
                            if ABLATE == "exponly":
                                continue
                            if g == 11 and normq:
                                nq_ = normq.pop(0)
                                bc = emit_recip(*nq_)
                                normq2.append((*nq_, bc))
                            # norm-mul early next block: the DRAM bounce (2
                            # DMA hops) lands by then, and the read completes
                            # long before this pv bank-half is re-used
                            if g == 1 and normq2:
                                nn = normq2.pop(0)
                                emit_norm(*nn)
                                normd[nn[1]] += 1
                            if g == 14 and deferred and normd[deferred[0]] == HEADS:
                                emit_outproj(deferred.pop(0))
                while pending:
                    pop_pv()
                while normq:
                    nq_ = normq.pop(0)
                    bc = emit_recip(*nq_)
                    normq2.append((*nq_, bc))
                while normq2:
                    nn = normq2.pop(0)
                    emit_norm(*nn)
                    normd[nn[1]] += 1
                while deferred:
                    ci_ = deferred.pop(0)
                    assert normd[ci_] == HEADS
                    emit_outproj(ci_)

                if debug:
                    nc.sync.dma_start(out=dbg["ksb"][:, :], in_=ksb[:])
                    nc.sync.dma_start(out=dbg["qsb0"][:, :], in_=qsb[0][:])
                    nc.sync.dma_start(
                        out=dbg["vsbT"][:, :], in_=vsbT[:].bitcast(U8)
                    )
                    nc.sync.dma_start(
                        out=dbg["outh0"][:, :], in_=outh[0][:].bitcast(F32)
                    )

    nc.finalize()
    return nc


_NC_CACHE = None


def make_in_maps(x, w_qkv, w_out, b_out):
    bf16 = ml_dtypes.bfloat16
    x = np.ascontiguousarray(np.asarray(x, dtype=np.float32)).reshape(4, C, N)
    w_qkv = np.asarray(w_qkv, dtype=np.float32)
    w_out = np.asarray(w_out, dtype=np.float32)
    b_out = np.asarray(b_out, dtype=np.float32)

    wq4 = np.ascontiguousarray((w_qkv[0:HID] * SCALE).T).astype(bf16)   # [256,128]
    wk4 = np.ascontiguousarray(w_qkv[HID:2 * HID].T).astype(bf16)
    wv4 = np.ascontiguousarray(w_qkv[2 * HID:3 * HID].T).astype(bf16)
    woT = np.ascontiguousarray(w_out.T)                                 # [128,256]
    boutc = np.ascontiguousarray(b_out.reshape(C, 1))
    xbf = x.astype(bf16)

    in_maps = []
    for core in range(NCORES):
        b, half = divmod(core, 2)
        in_maps.append(
            {
                "xb": xbf[b],
                "xq": np.ascontiguousarray(xbf[b][:, half * NQ:(half + 1) * NQ]),
                "wq4": wq4,
                "wk4": wk4,
                "wv4": wv4,
                "woT": woT,
                "bout": boutc,
            }
        )
    return in_maps


def kernel(x, w_qkv, w_out, b_out):
    global _NC_CACHE
    if _NC_CACHE is None:
        _NC_CACHE = build_nc()
    nc = _NC_CACHE
    in_maps = make_in_maps(x, w_qkv, w_out, b_out)
    res = run_bass_kernel_spmd(nc, in_maps, core_ids=list(range(NCORES)))
    out = np.empty((4, C, N), dtype=np.float32)
    for core in range(NCORES):
        b, half = divmod(core, 2)
        out[b][:, half * NQ:(half + 1) * NQ] = res.results[core]["out"]
    return out.reshape(4, C, 64, 64)


# revision 43
# speedup vs baseline: 1.1134x; 1.1134x over previous
"""Trainium2 Bass kernel for nn_Attention_21715354649378.

Reference computation (per batch b of 4):
    qkv = w_qkv @ x        x: [256, 4096(=64x64)]   w_qkv: [384, 256]
    q,k,v: [4 heads, 32, 4096];  q *= 32**-0.5
    sim_h = q_h^T k_h   [4096, 4096];  attn = softmax(sim, axis=-1)
    out_h = attn @ v_h^T    -> [4096, 32]
    out = w_out @ concat_heads + b_out   [256, 4096]

Sharding: 8 cores = 4 batches x 2 query-halves. Each core computes K/V for
its full batch plus attention + output projection for its half of the query
pixels. Outputs are disjoint slices -> no collectives.

The limiting resource (cost model): every sim element must transit
PSUM(fp32) -> SBUF through ACT or DVE at 1 elem/cycle/partition, so the
exp stage is organized as ONE fat [128, 1024] op per key-tile pair to
amortize the fixed access latency, with the work split A(CT)/S/T per the
EXP_PAT so ACT, DVE and Pool all stay busy:

  A: ACT activation Exp -> e4m3                      (exact + fp8 quant)
  T: DVE Schraudolph -> bf16 bits; PV runs bf16      (best, costs PE)
  S: DVE Schraudolph -> bf16 bits; Pool copy -> e4m3
  D: DVE Schraudolph -> e4m3 bits (uint8)            (coarse; unused)

Per-core algorithm (keys-in-partition layout; probs are exp(sim)*2^-4 so
they fit fp8-e4m3 range, the scale cancels in softmax normalization):

  ksb[32h+d, key] = W_k x, qsb[32h+d, q] = s W_q x_q   bf16
  vsbT = e4m3 v, keys in partitions, halves interleaved per key-tile pair
      for fp8 DoubleRow; a ones column per 48-block makes the PV matmul
      emit the softmax denominator as output row 32.

  per (h, ci) over 16 key-tile pairs:
    fat psum [128, 1024] <- 2 QK matmuls (contract 32, band 32h)
    probs [128, 1024] via one exp op (A on ACT / T,S on DVE, Pool converts S)
    pv[48, 512] += [v8|1|0..]^T probs   one fp8 DoubleRow matmul per pair
        (or 2 bf16 matmuls for T-pairs)
  norm: rec = recip(pv[32]) direct from PSUM; DRAM-bounce broadcast to 32
        partitions; outh[ci][32h..] = pv[0:32] * bc
  out[ci] = W_o @ outh[ci] + b_out -> DMA out

PSUM: 3x fat qk staging [128, 1024] (also used by projections / output
matmul staging) + 2x pv [128, 512].
"""

import os

import numpy as np
import ml_dtypes

ABLATE = os.environ.get("KERNEL_ABLATE", "")

import concourse.bass as bass
import concourse.mybir as mybir
import concourse.tile as tile
from concourse import bacc
from concourse.bass import ts, ds
from concourse.bass_utils import run_bass_kernel_spmd

HEADS = 4
D = 32
HID = 128
C = 256
N = 4096
NQ = 2048
SCALE = D ** -0.5
NCORES = 8

F32 = mybir.dt.float32
F32R = mybir.dt.float32r
BF16 = mybir.dt.bfloat16
E4 = mybir.dt.float8e4
I16 = mybir.dt.int16
U8 = mybir.dt.uint8
EXP = mybir.ActivationFunctionType.Exp
COPY = mybir.ActivationFunctionType.Copy
IDENT = mybir.ActivationFunctionType.Identity
DR = mybir.MatmulPerfMode.DoubleRow
MUL = mybir.AluOpType.mult
ADD = mybir.AluOpType.add

NKT = N // 128    # 32 key tiles
NPAIR = NKT // 2  # 16 key-tile pairs
NCH = NQ // 512   # 4 query chunks

# probs = exp(sim) * 2^-4  (fits e4m3; cancels in normalization)
LN2_4 = float(4 * np.log(2))
# Schraudolph bf16 bits: exp(x)*2^-4 ~= bf16(int16(x*184.665 + 15736))
SCH_A16 = 184.6650
SCH_B16 = 128.0 * (127 - 4) - 8.0
# Schraudolph e4m3 bits: exp(x)*2^-4 ~= e4m3(uint8(x*11.5416 + 23.6))
SCH_A8 = 11.541560
SCH_B8 = 8.0 * 3 - 0.4

# Engine split over all 256 pairs: A=130 (ACT), S=126 (DVE + casting-DMA
# convert). Strict near-alternation keeps the in-order PE / 3-slot PSUM
# rotation from chaining one engine's stalls into the other's.
def _mk_pat(nA, nS):
    out, a, s = [], 0, 0
    for _ in range(nA + nS):
        if a * nS <= s * nA and a < nA:
            out.append("A")
            a += 1
        else:
            out.append("S")
            s += 1
    return "".join(out)


EXP_PAT = _mk_pat(135, 121)
assert len(EXP_PAT) == 256
# PV trails its exp by a kind-dependent number of pairs: S-pairs wait out
# the bf16->e4m3 cast DMA (~3us); A-pair probs are ready right away.
PVLAG_A = 4
PVLAG_S = 9


def build_nc(debug=False):
    nc = bacc.Bacc("TRN2")

    dbg = {}
    if debug:
        dbg["ksb"] = nc.declare_dram_parameter("dbg_ksb", [128, N], BF16, isOutput=True)
        dbg["qsb0"] = nc.declare_dram_parameter("dbg_qsb0", [128, 512], BF16, isOutput=True)
        dbg["vsbT"] = nc.declare_dram_parameter("dbg_vsbT", [128, NPAIR * 384], U8, isOutput=True)
        dbg["probs0"] = nc.declare_dram_parameter("dbg_probs0", [128, 1024], U8, isOutput=True)
        dbg["pv0"] = nc.declare_dram_parameter("dbg_pv0", [128, 512], F32, isOutput=True)
        dbg["outh0"] = nc.declare_dram_parameter("dbg_outh0", [HID, 512], F32, isOutput=True)

    xb = nc.declare_dram_parameter("xb", [C, N], BF16, isOutput=False)
    xq = nc.declare_dram_parameter("xq", [C, NQ], BF16, isOutput=False)
    wq4 = nc.declare_dram_parameter("wq4", [C, HID], BF16, isOutput=False)
    wk4 = nc.declare_dram_parameter("wk4", [C, HID], BF16, isOutput=False)
    wv4 = nc.declare_dram_parameter("wv4", [C, HID], BF16, isOutput=False)
    woT = nc.declare_dram_parameter("woT", [HID, C], F32R, isOutput=False)
    bout = nc.declare_dram_parameter("bout", [C, 1], F32, isOutput=False)
    out = nc.declare_dram_parameter("out", [C, NQ], F32, isOutput=True)

    with tile.TileContext(nc) as tc:
        with (
            nc.allow_low_precision(reason="bf16 qk / fp8 pv attention core"),
            tc.tile_pool(name="persist", bufs=1) as persist,
            tc.tile_pool(name="wts", bufs=1) as wts,
        ):
            # ---- persistent SBUF ----
            x_sb = [
                [
                    persist.tile([128, 1024], BF16, tag=f"x{i}{j}", name=f"x{i}{j}")
                    for j in range(4)
                ]
                for i in range(2)
            ]
            xq_sb = [
                [
                    persist.tile([128, 1024], BF16, tag=f"xq{i}{j}", name=f"xq{i}{j}")
                    for j in range(2)
                ]
                for i in range(2)
            ]
            ksb = persist.tile([128, N], BF16, tag="ksb", name="ksb")
            qsb = [
                persist.tile([128, 512], BF16, tag=f"qsb{ci}", name=f"qsb{ci}")
                for ci in range(NCH)
            ]
            # per pair: 4 heads x 2 halves x 48 cols ([v8 (32) | ones | 0s]);
            # single tensor so V evacuations can write 4 pairs in one op.
            # DR stationary M must be a multiple of 16; engine PSUM reads
            # must start 32-aligned, so the denominator row sits at row 32.
            vsbT = persist.tile([128, NPAIR * 384], E4, tag="vsbT", name="vsbT")

            wq_sb = [
                wts.tile([128, HID], BF16, tag=f"wq{i}", name=f"wq{i}")
                for i in range(2)
            ]
            wk_sb = [
                wts.tile([128, HID], BF16, tag=f"wk{i}", name=f"wk{i}")
                for i in range(2)
            ]
            wv_sb = [
                wts.tile([128, HID], BF16, tag=f"wv{i}", name=f"wv{i}")
                for i in range(2)
            ]
            wo_sb = wts.tile([HID, C], F32R, tag="wo")
            bo_sb = [
                wts.tile([128, 1], F32, tag=f"bo{i}", name=f"bo{i}")
                for i in range(2)
            ]
            ebias = wts.tile([128, 1], F32, tag="ebias")

            # ---- input DMAs, ordered so K/Q projections can start earliest ----
            for i in range(2):
                nc.sync.dma_start(out=wk_sb[i][:], in_=wk4[ds(i * 128, 128), :])
            for i in range(2):
                nc.sync.dma_start(
                    out=x_sb[i][0][:], in_=xb[ds(i * 128, 128), ts(0, 1024)]
                )
            for i in range(2):
                nc.sync.dma_start(out=wq_sb[i][:], in_=wq4[ds(i * 128, 128), :])
            for i in range(2):
                nc.sync.dma_start(
                    out=xq_sb[i][0][:], in_=xq[ds(i * 128, 128), ts(0, 1024)]
                )
            for i in range(2):
                nc.sync.dma_start(out=wv_sb[i][:], in_=wv4[ds(i * 128, 128), :])
            for j in range(1, 4):
                for i in range(2):
                    nc.sync.dma_start(
                        out=x_sb[i][j][:], in_=xb[ds(i * 128, 128), ts(j, 1024)]
                    )
            for i in range(2):
                nc.sync.dma_start(
                    out=xq_sb[i][1][:], in_=xq[ds(i * 128, 128), ts(1, 1024)]
                )
            nc.sync.dma_start(out=wo_sb[:], in_=woT[:, :])
            for i in range(2):
                nc.sync.dma_start(out=bo_sb[i][:], in_=bout[ds(i * 128, 128), :])
            nc.vector.memset(ebias[:], -LN2_4)
            # v8 copies cover cols 0:32 of each 48-block; init only the
            # ones (col 32, denominator row) and zero-pad (cols 33:48)
            blk = vsbT[:].rearrange("q (p h hf x) -> q p h hf x", h=4, hf=2, x=48)
            nc.gpsimd.memset(blk[:, :, :, :, 32:33], 1.0)
            nc.gpsimd.memset(blk[:, :, :, :, 33:48], 0.0)

            def x_ap(ct, c0, length):
                t = c0 // 1024
                return x_sb[ct][t][:, ds(c0 % 1024, length)]

            def xq_ap(ct, c0, length):
                t = c0 // 1024
                return xq_sb[ct][t][:, ds(c0 % 1024, length)]

            with (
                tc.tile_pool(name="qkp", bufs=3, space="PSUM") as qkp,
                tc.tile_pool(name="pvp", bufs=1, space="PSUM") as pvp,
                tc.tile_pool(name="probs", bufs=22) as probs_pool,
                tc.tile_pool(name="norm", bufs=4) as norm_pool,
                tc.tile_pool(name="osb", bufs=2) as osb,
                tc.tile_pool(name="dram", bufs=2, space="DRAM") as dram_pool,
            ):
                def fat_tile():
                    return qkp.tile([128, 1024], F32, tag="qk", name="qk")

                # ---- projections (ride the fat psum rotation) ----
                def emit_k2(j2):  # j2 in 0..3, 1024-key chunk
                    ps = fat_tile()
                    for half in range(2):
                        for ct in range(2):
                            nc.tensor.matmul(
                                ps[:, ts(half, 512)],
                                wk_sb[ct][:],
                                x_ap(ct, (2 * j2 + half) * 512, 512),
                                start=(ct == 0),
                                stop=(ct == 1),
                            )
                    nc.scalar.activation(ksb[:, ts(j2, 1024)], ps[:], COPY)

                def emit_q(ci):
                    ps = fat_tile()[:, 0:512]
                    for ct in range(2):
                        nc.tensor.matmul(
                            ps,
                            wq_sb[ct][:],
                            xq_ap(ct, ci * 512, 512),
                            start=(ct == 0),
                            stop=(ct == 1),
                        )
                    nc.scalar.activation(qsb[ci][:], ps, COPY)

                def emit_v4(q0):  # quad q0 in 0..3: key tiles 8q0..8q0+7
                    ps = fat_tile()
                    for t in range(8):
                        for ct in range(2):
                            nc.tensor.matmul(
                                ps[:, ts(t, HID)],
                                x_ap(ct, (8 * q0 + t) * 128, 128),
                                wv_sb[ct][:],
                                start=(ct == 0),
                                stop=(ct == 1),
                            )
                    # evac: psum [kt(4 pairs x 2 hf), h, d] -> vsbT, one op
                    # per hf half (ISA engine APs allow at most 3 free dims)
                    src5 = ps[:].rearrange(
                        "q (tp thf h x) -> q tp thf h x", tp=4, thf=2, h=4
                    )
                    dst5 = vsbT[
                        :, ds(q0 * 4 * 384, 4 * 384)
                    ].rearrange(
                        "q (tp h thf x) -> q tp thf h x", tp=4, h=4, thf=2
                    )[:, :, :, :, 0:32]
                    for hf in range(2):
                        nc.scalar.activation(
                            dst5[:, :, hf], src5[:, :, hf], COPY
                        )

                outh = [
                    osb.tile([HID, 512], F32R, tag=f"outh{c}", name=f"outh{c}")
                    for c in range(NCH)
                ]

                # ---- exp paths (one fat op per pair) ----
                def exp_alloc(kind):
                    if kind == "A":
                        return probs_pool.tile([128, 1024], E4, tag="pr", name="prA")
                    if kind == "D":
                        return probs_pool.tile([128, 1024], U8, tag="pr", name="prD")
                    return probs_pool.tile([128, 1024], I16, tag="pr", name="prT")

                def exp_op(kind, pr, fat):
                    if kind == "A":
                        nc.scalar.activation(pr[:], fat[:], EXP, bias=ebias[:, 0:1])
                    elif kind == "D":
                        nc.vector.tensor_scalar(
                            pr[:], fat[:], SCH_A8, SCH_B8, MUL, ADD
                        )
                    else:
                        nc.vector.tensor_scalar(
                            pr[:], fat[:], SCH_A16, SCH_B16, MUL, ADD
                        )

                def exp_fini(kind, pb):
                    if kind != "S":
                        return pb
                    # S: SWDGE casting DMA converts the bf16 bits to e4m3
                    # (SBUF->SBUF; only gpsimd-initiated DMAs can cast)
                    pr = probs_pool.tile([128, 1024], E4, tag="pr", name="prS")
                    nc.gpsimd.dma_start(out=pr[:], in_=pb.bitcast(BF16)[:])
                    return pr

                # ---- normalization ----
                def emit_recip(h, ci, pv):
                    # den row PSUM->SBUF on ACT (balances DVE norm work),
                    # recip on DVE, then DRAM-bounce broadcast of 1/den.
                    r0, c0 = pv
                    den = norm_pool.tile([1, 512], F32, tag="den", name="den")
                    nc.scalar.activation(
                        den[:], pvar[ds(r0 + 32, 1), ds(c0, 512)], COPY
                    )
                    rec = norm_pool.tile([1, 512], F32, tag="rec", name="rec")
                    nc.vector.reciprocal_approx_fast(rec[:], den[:])
                    rdr = dram_pool.tile([1, 512], F32, tag="rdr", name="rdr")
                    nc.sync.dma_start(out=rdr[:], in_=rec[:])
                    bc = norm_pool.tile([D, 512], F32, tag="bc", name="bc")
                    nc.sync.dma_start(
                        out=bc[:],
                        in_=bass.AP(
                            tensor=rdr.tensor,
                            offset=rdr.offset,
                            ap=[[0, D]] + [list(a) for a in rdr.ap[1:]],
                        ),
                    )
                    return bc

                def emit_norm(h, ci, pv, bc):
                    # evacuation fused with normalization: outh = pv * (1/den)
                    r0, c0 = pv
                    nc.vector.tensor_mul(
                        outh[ci][ds(32 * h, 32), :],
                        pvar[ds(r0, 32), ds(c0, 512)],
                        bc[:],
                    )

                def emit_outproj(ci):
                    op = fat_tile()
                    for ot in range(2):
                        nc.tensor.matmul(
                            op[:, ts(ot, 512)],
                            wo_sb[:, ts(ot, 128)],
                            outh[ci][:],
                            start=True,
                            stop=True,
                        )
                    for ot in range(2):
                        ob = osb.tile([128, 512], F32, tag="ob", name="ob")
                        nc.scalar.activation(
                            ob[:], op[:, ts(ot, 512)], IDENT, bias=bo_sb[ot][:, 0:1]
                        )
                        nc.sync.dma_start(
                            out=out[ds(ot * 128, 128), ts(ci, 512)], in_=ob[:]
                        )

                # ---- prologue ----
                emit_k2(0)
                emit_q(0)
                emit_k2(1)
                emit_v4(0)
                emit_k2(2)
                emit_k2(3)
                vdone = 1

                pendA = []     # (idx, kind, probs, pair, h, ci, pv)
                pendS = []
                deferred = []  # ci ready for out-projection
                normq = []     # (h, ci, pv) pending denominator recip
                normq2 = []    # (h, ci, pv, bc) pending normalization
                normd = [0] * NCH  # per-ci count of emitted norm-muls
                popped = {}    # (h, ci) -> number of pairs PV'd so far

                def pop_one(ent):
                    _, kind, probs, p, h, ci, pv = ent
                    r0, c0 = pv
                    pvap = pvar[ds(r0, 48), ds(c0, 512)]
                    vv = vsbT[:, ds(384 * p + 96 * h, 96)]
                    # start/stop by EMISSION order (pops run out of pair
                    # order): first emitted clears PSUM, 16th closes group
                    npop = popped.get((h, ci), 0)
                    first, last = npop == 0, npop == NPAIR - 1
                    popped[(h, ci)] = npop + 1
                    if kind == "T":
                        # bf16 probs: one plain matmul per key tile
                        prb = probs.bitcast(BF16)
                        for t in range(2):
                            nc.tensor.matmul(
                                pvap,
                                vv[:, ds(48 * t, 48)],
                                prb[:, ts(t, 512)],
                                start=(first and t == 0),
                                stop=(last and t == 1),
                            )
                    else:
                        prb = probs.bitcast(E4) if kind == "D" else probs
                        nc.tensor.matmul(
                            pvap,
                            vv.rearrange("q (hf m) -> q hf m", hf=2),
                            prb[:].rearrange("q (hf n) -> q hf n", hf=2),
                            start=first,
                            stop=last,
                            perf_mode=DR,
                        )
                    if last and ABLATE != "nonorm":
                        normq.append((h, ci, pv))
                        if h == HEADS - 1:
                            deferred.append(ci)

                def pop_ready(gp):
                    while pendA and gp - pendA[0][0] >= PVLAG_A:
                        pop_one(pendA.pop(0))
                    while pendS and gp - pendS[0][0] >= PVLAG_S:
                        pop_one(pendS.pop(0))

                # pv arena: 2 banks, alternating per block; with recip at
                # g11 and norm-mul at g1 of the next block, a block's pv is
                # fully read ~10 pairs before its bank is re-written.
                pvar = pvp.tile([128, 1024], F32, tag="pvar", name="pvar")

                _gexp = [0]
                for h in range(HEADS):
                    for ci in range(NCH):
                        blk = h * NCH + ci
                        pv = (0, 512 * (blk % 2))
                        for g in range(NPAIR):
                            kind = EXP_PAT[_gexp[0] % len(EXP_PAT)]
                            _gexp[0] += 1
                            pr0 = exp_alloc(kind)
                            fat = fat_tile()
                            for t in range(2):
                                kt = 2 * g + t
                                nc.tensor.matmul(
                                    fat[:, ts(t, 512)],
                                    ksb[ds(32 * h, 32), ts(kt, 128)],
                                    qsb[ci][ds(32 * h, 32), :],
                                    start=True,
                                    stop=True,
                                    tile_position=(32 * h, 0),
                                )
                            exp_op(kind, pr0, fat)
                            pr = exp_fini(kind, pr0)
                            if debug and h == 0 and ci == 0 and g == 0:
                                nc.sync.dma_start(
                                    out=dbg["probs0"][:, :],
                                    in_=pr[:].bitcast(U8),
                                )
                            gp = _gexp[0] - 1
                            if ABLATE != "exponly":
                                ent = (gp, kind, pr, g, h, ci, pv)
                                (pendA if kind != "S" else pendS).append(ent)
                                pop_ready(gp)
                            # interleave deferred work into the PE stream
                            if g % 2 == 1 and vdone < 4:
                                emit_v4(vdone)
                                vdone += 1
                            if g == 2 and h == 0 and ci < NCH - 1:
                                emit_q(ci + 1)
                            if ABLATE == "exponly":
                                continue
                            if g == 11 and normq:
                                nq_ = normq.pop(0)
                                bc = emit_recip(*nq_)
                                normq2.append((*nq_, bc))
                            # norm-mul early next block: the DRAM bounce (2
                            # DMA hops) lands by then, and the read completes
                            # long before this pv bank-half is re-used
                            if g == 1 and normq2:
                                nn = normq2.pop(0)
                                emit_norm(*nn)
                                normd[nn[1]] += 1
                            if g == 14 and deferred and normd[deferred[0]] == HEADS:
                                emit_outproj(deferred.pop(0))
                while pendA or pendS:
                    if pendA:
                        pop_one(pendA.pop(0))
                    if pendS:
                        pop_one(pendS.pop(0))
                while normq:
                    nq_ = normq.pop(0)
                    bc = emit_recip(*nq_)
                    normq2.append((*nq_, bc))
                while normq2:
                    nn = normq2.pop(0)
                    emit_norm(*nn)
                    normd[nn[1]] += 1
                while deferred:
                    ci_ = deferred.pop(0)
                    assert normd[ci_] == HEADS
                    emit_outproj(ci_)

                if debug:
                    nc.sync.dma_start(out=dbg["ksb"][:, :], in_=ksb[:])
                    nc.sync.dma_start(out=dbg["qsb0"][:, :], in_=qsb[0][:])
                    nc.sync.dma_start(
                        out=dbg["vsbT"][:, :], in_=vsbT[:].bitcast(U8)
                    )
                    nc.sync.dma_start(
                        out=dbg["outh0"][:, :], in_=outh[0][:].bitcast(F32)
                    )

    nc.finalize()
    return nc


_NC_CACHE = None


def make_in_maps(x, w_qkv, w_out, b_out):
    bf16 = ml_dtypes.bfloat16
    x = np.ascontiguousarray(np.asarray(x, dtype=np.float32)).reshape(4, C, N)
    w_qkv = np.asarray(w_qkv, dtype=np.float32)
    w_out = np.asarray(w_out, dtype=np.float32)
    b_out = np.asarray(b_out, dtype=np.float32)

    wq4 = np.ascontiguousarray((w_qkv[0:HID] * SCALE).T).astype(bf16)   # [256,128]
    wk4 = np.ascontiguousarray(w_qkv[HID:2 * HID].T).astype(bf16)
    wv4 = np.ascontiguousarray(w_qkv[2 * HID:3 * HID].T).astype(bf16)
    woT = np.ascontiguousarray(w_out.T)                                 # [128,256]
    boutc = np.ascontiguousarray(b_out.reshape(C, 1))
    xbf = x.astype(bf16)

    in_maps = []
    for core in range(NCORES):
        b, half = divmod(core, 2)
        in_maps.append(
            {
                "xb": xbf[b],
                "xq": np.ascontiguousarray(xbf[b][:, half * NQ:(half + 1) * NQ]),
                "wq4": wq4,
                "wk4": wk4,
                "wv4": wv4,
                "woT": woT,
                "bout": boutc,
            }
        )
    return in_maps


def kernel(x, w_qkv, w_out, b_out):
    global _NC_CACHE
    if _NC_CACHE is None:
        _NC_CACHE = build_nc()
    nc = _NC_CACHE
    in_maps = make_in_maps(x, w_qkv, w_out, b_out)
    res = run_bass_kernel_spmd(nc, in_maps, core_ids=list(range(NCORES)))
    out = np.empty((4, C, N), dtype=np.float32)
    for core in range(NCORES):
        b, half = divmod(core, 2)
        out[b][:, half * NQ:(half + 1) * NQ] = res.results[core]["out"]
    return out.reshape(4, C, 64, 64)


# revision 44
# speedup vs baseline: 1.1397x; 1.0236x over previous
"""Trainium2 Bass kernel for nn_Attention_21715354649378.

Reference computation (per batch b of 4):
    qkv = w_qkv @ x        x: [256, 4096(=64x64)]   w_qkv: [384, 256]
    q,k,v: [4 heads, 32, 4096];  q *= 32**-0.5
    sim_h = q_h^T k_h   [4096, 4096];  attn = softmax(sim, axis=-1)
    out_h = attn @ v_h^T    -> [4096, 32]
    out = w_out @ concat_heads + b_out   [256, 4096]

Sharding: 8 cores = 4 batches x 2 query-halves. Each core computes K/V for
its full batch plus attention + output projection for its half of the query
pixels. Outputs are disjoint slices -> no collectives.

The limiting resource (cost model): every sim element must transit
PSUM(fp32) -> SBUF through ACT or DVE at 1 elem/cycle/partition, so the
exp stage is organized as ONE fat [128, 1024] op per key-tile pair to
amortize the fixed access latency, with the work split A(CT)/S/T per the
EXP_PAT so ACT, DVE and Pool all stay busy:

  A: ACT activation Exp -> e4m3                      (exact + fp8 quant)
  T: DVE Schraudolph -> bf16 bits; PV runs bf16      (best, costs PE)
  S: DVE Schraudolph -> bf16 bits; Pool copy -> e4m3
  D: DVE Schraudolph -> e4m3 bits (uint8)            (coarse; unused)

Per-core algorithm (keys-in-partition layout; probs are exp(sim)*2^-4 so
they fit fp8-e4m3 range, the scale cancels in softmax normalization):

  ksb[32h+d, key] = W_k x, qsb[32h+d, q] = s W_q x_q   bf16
  vsbT = e4m3 v, keys in partitions, halves interleaved per key-tile pair
      for fp8 DoubleRow; a ones column per 48-block makes the PV matmul
      emit the softmax denominator as output row 32.

  per (h, ci) over 16 key-tile pairs:
    fat psum [128, 1024] <- 2 QK matmuls (contract 32, band 32h)
    probs [128, 1024] via one exp op (A on ACT / T,S on DVE, Pool converts S)
    pv[48, 512] += [v8|1|0..]^T probs   one fp8 DoubleRow matmul per pair
        (or 2 bf16 matmuls for T-pairs)
  norm: rec = recip(pv[32]) direct from PSUM; DRAM-bounce broadcast to 32
        partitions; outh[ci][32h..] = pv[0:32] * bc
  out[ci] = W_o @ outh[ci] + b_out -> DMA out

PSUM: 3x fat qk staging [128, 1024] (also used by projections / output
matmul staging) + 2x pv [128, 512].
"""

import os

import numpy as np
import ml_dtypes

ABLATE = os.environ.get("KERNEL_ABLATE", "")

import concourse.bass as bass
import concourse.mybir as mybir
import concourse.tile as tile
from concourse import bacc
from concourse.bass import ts, ds
from concourse.bass_utils import run_bass_kernel_spmd

HEADS = 4
D = 32
HID = 128
C = 256
N = 4096
NQ = 2048
SCALE = D ** -0.5
NCORES = 8

F32 = mybir.dt.float32
F32R = mybir.dt.float32r
BF16 = mybir.dt.bfloat16
E4 = mybir.dt.float8e4
I16 = mybir.dt.int16
U8 = mybir.dt.uint8
EXP = mybir.ActivationFunctionType.Exp
COPY = mybir.ActivationFunctionType.Copy
IDENT = mybir.ActivationFunctionType.Identity
DR = mybir.MatmulPerfMode.DoubleRow
MUL = mybir.AluOpType.mult
ADD = mybir.AluOpType.add

NKT = N // 128    # 32 key tiles
NPAIR = NKT // 2  # 16 key-tile pairs
NCH = NQ // 512   # 4 query chunks

# probs = exp(sim) * 2^-4  (fits e4m3; cancels in normalization)
LN2_4 = float(4 * np.log(2))
# Schraudolph bf16 bits: exp(x)*2^-4 ~= bf16(int16(x*184.665 + 15736))
SCH_A16 = 184.6650
SCH_B16 = 128.0 * (127 - 4) - 8.0
# Schraudolph e4m3 bits: exp(x)*2^-4 ~= e4m3(uint8(x*11.5416 + 23.6))
SCH_A8 = 11.541560
SCH_B8 = 8.0 * 3 - 0.4

# Engine split over all 256 pairs: A=130 (ACT), S=126 (DVE + casting-DMA
# convert). Strict near-alternation keeps the in-order PE / 3-slot PSUM
# rotation from chaining one engine's stalls into the other's.
def _mk_pat(nA, nS):
    out, a, s = [], 0, 0
    for _ in range(nA + nS):
        if a * nS <= s * nA and a < nA:
            out.append("A")
            a += 1
        else:
            out.append("S")
            s += 1
    return "".join(out)


EXP_PAT = _mk_pat(129, 127)
assert len(EXP_PAT) == 256
# PV trails its exp by a kind-dependent number of pairs: S-pairs wait out
# the bf16->e4m3 cast DMA (~3us); A-pair probs are ready right away.
PVLAG_A = 3
PVLAG_S = 9


def build_nc(debug=False):
    nc = bacc.Bacc("TRN2")

    dbg = {}
    if debug:
        dbg["ksb"] = nc.declare_dram_parameter("dbg_ksb", [128, N], BF16, isOutput=True)
        dbg["qsb0"] = nc.declare_dram_parameter("dbg_qsb0", [128, 512], BF16, isOutput=True)
        dbg["vsbT"] = nc.declare_dram_parameter("dbg_vsbT", [128, NPAIR * 384], U8, isOutput=True)
        dbg["probs0"] = nc.declare_dram_parameter("dbg_probs0", [128, 1024], U8, isOutput=True)
        dbg["pv0"] = nc.declare_dram_parameter("dbg_pv0", [128, 512], F32, isOutput=True)
        dbg["outh0"] = nc.declare_dram_parameter("dbg_outh0", [HID, 512], F32, isOutput=True)

    xb = nc.declare_dram_parameter("xb", [C, N], BF16, isOutput=False)
    xq = nc.declare_dram_parameter("xq", [C, NQ], BF16, isOutput=False)
    wq4 = nc.declare_dram_parameter("wq4", [C, HID], BF16, isOutput=False)
    wk4 = nc.declare_dram_parameter("wk4", [C, HID], BF16, isOutput=False)
    wv4 = nc.declare_dram_parameter("wv4", [C, HID], BF16, isOutput=False)
    woT = nc.declare_dram_parameter("woT", [HID, C], F32R, isOutput=False)
    bout = nc.declare_dram_parameter("bout", [C, 1], F32, isOutput=False)
    out = nc.declare_dram_parameter("out", [C, NQ], F32, isOutput=True)

    with tile.TileContext(nc) as tc:
        with (
            nc.allow_low_precision(reason="bf16 qk / fp8 pv attention core"),
            tc.tile_pool(name="persist", bufs=1) as persist,
            tc.tile_pool(name="wts", bufs=1) as wts,
        ):
            # ---- persistent SBUF ----
            x_sb = [
                [
                    persist.tile([128, 1024], BF16, tag=f"x{i}{j}", name=f"x{i}{j}")
                    for j in range(4)
                ]
                for i in range(2)
            ]
            xq_sb = [
                [
                    persist.tile([128, 1024], BF16, tag=f"xq{i}{j}", name=f"xq{i}{j}")
                    for j in range(2)
                ]
                for i in range(2)
            ]
            ksb = persist.tile([128, N], BF16, tag="ksb", name="ksb")
            qsb = [
                persist.tile([128, 512], BF16, tag=f"qsb{ci}", name=f"qsb{ci}")
                for ci in range(NCH)
            ]
            # per pair: 4 heads x 2 halves x 48 cols ([v8 (32) | ones | 0s]);
            # single tensor so V evacuations can write 4 pairs in one op.
            # DR stationary M must be a multiple of 16; engine PSUM reads
            # must start 32-aligned, so the denominator row sits at row 32.
            vsbT = persist.tile([128, NPAIR * 384], E4, tag="vsbT", name="vsbT")

            wq_sb = [
                wts.tile([128, HID], BF16, tag=f"wq{i}", name=f"wq{i}")
                for i in range(2)
            ]
            wk_sb = [
                wts.tile([128, HID], BF16, tag=f"wk{i}", name=f"wk{i}")
                for i in range(2)
            ]
            wv_sb = [
                wts.tile([128, HID], BF16, tag=f"wv{i}", name=f"wv{i}")
                for i in range(2)
            ]
            wo_sb = wts.tile([HID, C], F32R, tag="wo")
            bo_sb = [
                wts.tile([128, 1], F32, tag=f"bo{i}", name=f"bo{i}")
                for i in range(2)
            ]
            ebias = wts.tile([128, 1], F32, tag="ebias")

            # ---- input DMAs, ordered so K/Q projections can start earliest ----
            for i in range(2):
                nc.sync.dma_start(out=wk_sb[i][:], in_=wk4[ds(i * 128, 128), :])
            for i in range(2):
                nc.sync.dma_start(
                    out=x_sb[i][0][:], in_=xb[ds(i * 128, 128), ts(0, 1024)]
                )
            for i in range(2):
                nc.sync.dma_start(out=wq_sb[i][:], in_=wq4[ds(i * 128, 128), :])
            for i in range(2):
                nc.sync.dma_start(
                    out=xq_sb[i][0][:], in_=xq[ds(i * 128, 128), ts(0, 1024)]
                )
            for i in range(2):
                nc.sync.dma_start(out=wv_sb[i][:], in_=wv4[ds(i * 128, 128), :])
            for j in range(1, 4):
                for i in range(2):
                    nc.sync.dma_start(
                        out=x_sb[i][j][:], in_=xb[ds(i * 128, 128), ts(j, 1024)]
                    )
            for i in range(2):
                nc.sync.dma_start(
                    out=xq_sb[i][1][:], in_=xq[ds(i * 128, 128), ts(1, 1024)]
                )
            nc.sync.dma_start(out=wo_sb[:], in_=woT[:, :])
            for i in range(2):
                nc.sync.dma_start(out=bo_sb[i][:], in_=bout[ds(i * 128, 128), :])
            nc.vector.memset(ebias[:], -LN2_4)
            # v8 copies cover cols 0:32 of each 48-block; init only the
            # ones (col 32, denominator row) and zero-pad (cols 33:48)
            blk = vsbT[:].rearrange("q (p h hf x) -> q p h hf x", h=4, hf=2, x=48)
            nc.gpsimd.memset(blk[:, :, :, :, 32:33], 1.0)
            nc.gpsimd.memset(blk[:, :, :, :, 33:48], 0.0)

            def x_ap(ct, c0, length):
                t = c0 // 1024
                return x_sb[ct][t][:, ds(c0 % 1024, length)]

            def xq_ap(ct, c0, length):
                t = c0 // 1024
                return xq_sb[ct][t][:, ds(c0 % 1024, length)]

            with (
                tc.tile_pool(name="qkp", bufs=3, space="PSUM") as qkp,
                tc.tile_pool(name="pvp", bufs=1, space="PSUM") as pvp,
                tc.tile_pool(name="probs", bufs=22) as probs_pool,
                tc.tile_pool(name="norm", bufs=4) as norm_pool,
                tc.tile_pool(name="osb", bufs=2) as osb,
                tc.tile_pool(name="dram", bufs=2, space="DRAM") as dram_pool,
            ):
                def fat_tile():
                    return qkp.tile([128, 1024], F32, tag="qk", name="qk")

                # ---- projections (ride the fat psum rotation) ----
                def emit_k2(j2):  # j2 in 0..3, 1024-key chunk
                    ps = fat_tile()
                    for half in range(2):
                        for ct in range(2):
                            nc.tensor.matmul(
                                ps[:, ts(half, 512)],
                                wk_sb[ct][:],
                                x_ap(ct, (2 * j2 + half) * 512, 512),
                                start=(ct == 0),
                                stop=(ct == 1),
                            )
                    nc.scalar.activation(ksb[:, ts(j2, 1024)], ps[:], COPY)

                def emit_q(ci):
                    ps = fat_tile()[:, 0:512]
                    for ct in range(2):
                        nc.tensor.matmul(
                            ps,
                            wq_sb[ct][:],
                            xq_ap(ct, ci * 512, 512),
                            start=(ct == 0),
                            stop=(ct == 1),
                        )
                    nc.scalar.activation(qsb[ci][:], ps, COPY)

                def emit_v4(q0):  # quad q0 in 0..3: key tiles 8q0..8q0+7
                    ps = fat_tile()
                    for t in range(8):
                        for ct in range(2):
                            nc.tensor.matmul(
                                ps[:, ts(t, HID)],
                                x_ap(ct, (8 * q0 + t) * 128, 128),
                                wv_sb[ct][:],
                                start=(ct == 0),
                                stop=(ct == 1),
                            )
                    # evac: psum [kt(4 pairs x 2 hf), h, d] -> vsbT, one op
                    # per hf half (ISA engine APs allow at most 3 free dims)
                    src5 = ps[:].rearrange(
                        "q (tp thf h x) -> q tp thf h x", tp=4, thf=2, h=4
                    )
                    dst5 = vsbT[
                        :, ds(q0 * 4 * 384, 4 * 384)
                    ].rearrange(
                        "q (tp h thf x) -> q tp thf h x", tp=4, h=4, thf=2
                    )[:, :, :, :, 0:32]
                    for hf in range(2):
                        nc.scalar.activation(
                            dst5[:, :, hf], src5[:, :, hf], COPY
                        )

                outh = [
                    osb.tile([HID, 512], F32R, tag=f"outh{c}", name=f"outh{c}")
                    for c in range(NCH)
                ]

                # ---- exp paths (one fat op per pair) ----
                def exp_alloc(kind):
                    if kind == "A":
                        return probs_pool.tile([128, 1024], E4, tag="pr", name="prA")
                    if kind == "D":
                        return probs_pool.tile([128, 1024], U8, tag="pr", name="prD")
                    return probs_pool.tile([128, 1024], I16, tag="pr", name="prT")

                def exp_op(kind, pr, fat):
                    if kind == "A":
                        nc.scalar.activation(pr[:], fat[:], EXP, bias=ebias[:, 0:1])
                    elif kind == "D":
                        nc.vector.tensor_scalar(
                            pr[:], fat[:], SCH_A8, SCH_B8, MUL, ADD
                        )
                    else:
                        nc.vector.tensor_scalar(
                            pr[:], fat[:], SCH_A16, SCH_B16, MUL, ADD
                        )

                def exp_fini(kind, pb):
                    if kind != "S":
                        return pb
                    # S: SWDGE casting DMA converts the bf16 bits to e4m3
                    # (SBUF->SBUF; only gpsimd-initiated DMAs can cast)
                    pr = probs_pool.tile([128, 1024], E4, tag="pr", name="prS")
                    nc.gpsimd.dma_start(out=pr[:], in_=pb.bitcast(BF16)[:])
                    return pr

                # ---- normalization ----
                def emit_recip(h, ci, pv):
                    # den row PSUM->SBUF on ACT (balances DVE norm work),
                    # recip on DVE, then DRAM-bounce broadcast of 1/den.
                    r0, c0 = pv
                    den = norm_pool.tile([1, 512], F32, tag="den", name="den")
                    nc.scalar.activation(
                        den[:], pvar[ds(r0 + 32, 1), ds(c0, 512)], COPY
                    )
                    rec = norm_pool.tile([1, 512], F32, tag="rec", name="rec")
                    nc.vector.reciprocal_approx_fast(rec[:], den[:])
                    rdr = dram_pool.tile([1, 512], F32, tag="rdr", name="rdr")
                    nc.sync.dma_start(out=rdr[:], in_=rec[:])
                    bc = norm_pool.tile([D, 512], F32, tag="bc", name="bc")
                    nc.sync.dma_start(
                        out=bc[:],
                        in_=bass.AP(
                            tensor=rdr.tensor,
                            offset=rdr.offset,
                            ap=[[0, D]] + [list(a) for a in rdr.ap[1:]],
                        ),
                    )
                    return bc

                def emit_norm(h, ci, pv, bc):
                    # evacuation fused with normalization: outh = pv * (1/den)
                    r0, c0 = pv
                    nc.vector.tensor_mul(
                        outh[ci][ds(32 * h, 32), :],
                        pvar[ds(r0, 32), ds(c0, 512)],
                        bc[:],
                    )

                def emit_outproj(ci):
                    op = fat_tile()
                    for ot in range(2):
                        nc.tensor.matmul(
                            op[:, ts(ot, 512)],
                            wo_sb[:, ts(ot, 128)],
                            outh[ci][:],
                            start=True,
                            stop=True,
                        )
                    for ot in range(2):
                        ob = osb.tile([128, 512], F32, tag="ob", name="ob")
                        nc.scalar.activation(
                            ob[:], op[:, ts(ot, 512)], IDENT, bias=bo_sb[ot][:, 0:1]
                        )
                        nc.sync.dma_start(
                            out=out[ds(ot * 128, 128), ts(ci, 512)], in_=ob[:]
                        )

                # ---- prologue ----
                emit_k2(0)
                emit_q(0)
                emit_k2(1)
                emit_v4(0)
                emit_k2(2)
                emit_k2(3)
                vdone = 1

                pendA = []     # (idx, kind, probs, pair, h, ci, pv)
                pendS = []
                deferred = []  # ci ready for out-projection
                normq = []     # (h, ci, pv) pending denominator recip
                normq2 = []    # (h, ci, pv, bc) pending normalization
                normd = [0] * NCH  # per-ci count of emitted norm-muls
                popped = {}    # (h, ci) -> number of pairs PV'd so far

                def pop_one(ent):
                    _, kind, probs, p, h, ci, pv = ent
                    r0, c0 = pv
                    pvap = pvar[ds(r0, 48), ds(c0, 512)]
                    vv = vsbT[:, ds(384 * p + 96 * h, 96)]
                    # start/stop by EMISSION order (pops run out of pair
                    # order): first emitted clears PSUM, 16th closes group
                    npop = popped.get((h, ci), 0)
                    first, last = npop == 0, npop == NPAIR - 1
                    popped[(h, ci)] = npop + 1
                    if kind == "T":
                        # bf16 probs: one plain matmul per key tile
                        prb = probs.bitcast(BF16)
                        for t in range(2):
                            nc.tensor.matmul(
                                pvap,
                                vv[:, ds(48 * t, 48)],
                                prb[:, ts(t, 512)],
                                start=(first and t == 0),
                                stop=(last and t == 1),
                            )
                    else:
                        prb = probs.bitcast(E4) if kind == "D" else probs
                        nc.tensor.matmul(
                            pvap,
                            vv.rearrange("q (hf m) -> q hf m", hf=2),
                            prb[:].rearrange("q (hf n) -> q hf n", hf=2),
                            start=first,
                            stop=last,
                            perf_mode=DR,
                        )
                    if last and ABLATE != "nonorm":
                        normq.append((h, ci, pv))
                        if h == HEADS - 1:
                            deferred.append(ci)

                def pop_ready(gp):
                    while pendA and gp - pendA[0][0] >= PVLAG_A:
                        pop_one(pendA.pop(0))
                    while pendS and gp - pendS[0][0] >= PVLAG_S:
                        pop_one(pendS.pop(0))

                # pv arena: 2 banks, alternating per block; with recip at
                # g11 and norm-mul at g1 of the next block, a block's pv is
                # fully read ~10 pairs before its bank is re-written.
                pvar = pvp.tile([128, 1024], F32, tag="pvar", name="pvar")

                _gexp = [0]
                for h in range(HEADS):
                    for ci in range(NCH):
                        blk = h * NCH + ci
                        pv = (0, 512 * (blk % 2))
                        for g in range(NPAIR):
                            kind = EXP_PAT[_gexp[0] % len(EXP_PAT)]
                            _gexp[0] += 1
                            pr0 = exp_alloc(kind)
                            fat = fat_tile()
                            for t in range(2):
                                kt = 2 * g + t
                                nc.tensor.matmul(
                                    fat[:, ts(t, 512)],
                                    ksb[ds(32 * h, 32), ts(kt, 128)],
                                    qsb[ci][ds(32 * h, 32), :],
                                    start=True,
                                    stop=True,
                                    tile_position=(32 * h, 0),
                                )
                            exp_op(kind, pr0, fat)
                            pr = exp_fini(kind, pr0)
                            if debug and h == 0 and ci == 0 and g == 0:
                                nc.sync.dma_start(
                                    out=dbg["probs0"][:, :],
                                    in_=pr[:].bitcast(U8),
                                )
                            gp = _gexp[0] - 1
                            if ABLATE != "exponly":
                                ent = (gp, kind, pr, g, h, ci, pv)
                                (pendA if kind != "S" else pendS).append(ent)
                                pop_ready(gp)
                            # interleave deferred work into the PE stream
                            if g % 2 == 1 and vdone < 4:
                                emit_v4(vdone)
                                vdone += 1
                            if g == 2 and h == 0 and ci < NCH - 1:
                                emit_q(ci + 1)
                            if ABLATE == "exponly":
                                continue
                            if g == 11 and normq:
                                nq_ = normq.pop(0)
                                bc = emit_recip(*nq_)
                                normq2.append((*nq_, bc))
                            # norm-mul early next block: the DRAM bounce (2
                            # DMA hops) lands by then, and the read completes
                            # long before this pv bank-half is re-used
                            if g == 1 and normq2:
                                nn = normq2.pop(0)
                                emit_norm(*nn)
                                normd[nn[1]] += 1
                            if g == 14 and deferred and normd[deferred[0]] == HEADS:
                                emit_outproj(deferred.pop(0))
                while pendA or pendS:
                    if pendA:
                        pop_one(pendA.pop(0))
                    if pendS:
                        pop_one(pendS.pop(0))
                while normq:
                    nq_ = normq.pop(0)
                    bc = emit_recip(*nq_)
                    normq2.append((*nq_, bc))
                while normq2:
                    nn = normq2.pop(0)
                    emit_norm(*nn)
                    normd[nn[1]] += 1
                while deferred:
                    ci_ = deferred.pop(0)
                    assert normd[ci_] == HEADS
                    emit_outproj(ci_)

                if debug:
                    nc.sync.dma_start(out=dbg["ksb"][:, :], in_=ksb[:])
                    nc.sync.dma_start(out=dbg["qsb0"][:, :], in_=qsb[0][:])
                    nc.sync.dma_start(
                        out=dbg["vsbT"][:, :], in_=vsbT[:].bitcast(U8)
                    )
                    nc.sync.dma_start(
                        out=dbg["outh0"][:, :], in_=outh[0][:].bitcast(F32)
                    )

    nc.finalize()
    return nc


_NC_CACHE = None


def make_in_maps(x, w_qkv, w_out, b_out):
    bf16 = ml_dtypes.bfloat16
    x = np.ascontiguousarray(np.asarray(x, dtype=np.float32)).reshape(4, C, N)
    w_qkv = np.asarray(w_qkv, dtype=np.float32)
    w_out = np.asarray(w_out, dtype=np.float32)
    b_out = np.asarray(b_out, dtype=np.float32)

    wq4 = np.ascontiguousarray((w_qkv[0:HID] * SCALE).T).astype(bf16)   # [256,128]
    wk4 = np.ascontiguousarray(w_qkv[HID:2 * HID].T).astype(bf16)
    wv4 = np.ascontiguousarray(w_qkv[2 * HID:3 * HID].T).astype(bf16)
    woT = np.ascontiguousarray(w_out.T)                                 # [128,256]
    boutc = np.ascontiguousarray(b_out.reshape(C, 1))
    xbf = x.astype(bf16)

    in_maps = []
    for core in range(NCORES):
        b, half = divmod(core, 2)
        in_maps.append(
            {
                "xb": xbf[b],
                "xq": np.ascontiguousarray(xbf[b][:, half * NQ:(half + 1) * NQ]),
                "wq4": wq4,
                "wk4": wk4,
                "wv4": wv4,
                "woT": woT,
                "bout": boutc,
            }
        )
    return in_maps


def kernel(x, w_qkv, w_out, b_out):
    global _NC_CACHE
    if _NC_CACHE is None:
        _NC_CACHE = build_nc()
    nc = _NC_CACHE
    in_maps = make_in_maps(x, w_qkv, w_out, b_out)
    res = run_bass_kernel_spmd(nc, in_maps, core_ids=list(range(NCORES)))
    out = np.empty((4, C, N), dtype=np.float32)
    for core in range(NCORES):
        b, half = divmod(core, 2)
        out[b][:, half * NQ:(half + 1) * NQ] = res.results[core]["out"]
    return out.reshape(4, C, 64, 64)
